# revision 13
# baseline (speedup 1.0000x reference)
"""Trainium2 Bass kernel for a dense transformer block (B=2, T=2048, C=1024,
NH=16, HD=64, FF=4x), distributed over 8 NeuronCores.

Sharding: data-parallel over batch (2 groups of 4 cores) x tensor-parallel over
heads within a group (4 heads/core), with sequence-parallel FFN: attention
output partials are ReduceScattered over T inside each group as 4 per-band
collectives (512 rows each), then each core runs LN2+FFN on its own 512 rows.

All matmul operands are fp16 (error gate is 2e-2; fp16 keeps ~3e-4).
LayerNorm stats, PSUM accumulation and the residual stream stay fp32.
Transposes run on the DMA XBAR, split across the two HWDGE queues (sync +
scalar) so two halves proceed in parallel.  The gpsimd queue is reserved for
the collectives so their doorbells never sit behind blocked DMA dispatches.
Score matmuls (K=64) for the two heads of a pair are emitted adjacently so
they run concurrently on PE row-groups 0-63 / 64-127.
LN gains are folded into the weights host-side (exact algebra):
  xn = g*z + be  (z = (x-mean)/std)  =>  xn @ W = z @ (g*W) + be @ W
"""

import contextlib
import os
import sys
import types

import numpy as np

# --- NTFF profile hook shim (tracing support; harmless when unused) ---------
def _install_ntff_hook_shim():
    if "antenv.axon_hooks" in sys.modules:
        return
    try:
        import antenv
        import trn_agent_boot.trn_boot as tb

        mod = types.ModuleType("antenv.axon_hooks")
        holder = [None]
        mod.set_axon_ntff_profile_hook = lambda h: holder.__setitem__(0, h)
        mod.get_axon_ntff_profile_hook = lambda: holder[0]
        sys.modules["antenv.axon_hooks"] = mod
        antenv.axon_hooks = mod
        if os.path.exists("/opt/axon/libaxon_pjrt.so"):
            mod.set_axon_ntff_profile_hook(
                tb._ntff_profile_via_ctypes("/opt/axon/libaxon_pjrt.so")
            )
    except Exception:
        pass


_install_ntff_hook_shim()

import concourse.bass as bass
import concourse.mybir as mybir
import concourse.tile as tile
from concourse.tile_rust import add_dep_helper
from concourse import bacc
from concourse import bass_utils

# Problem shape (hardcoded per contest rules).
B, T, C, NH, HD = 2, 2048, 1024, 16, 64
FF = 4 * C  # 4096
EPS = 1e-6
P = 128
NCORES = 8
TPG = 4            # cores per batch group
NHL = NH // TPG    # local heads per core = 4
TLOC = T // TPG    # rows per core after ReduceScatter = 512
KO = C // P        # 8 contraction chunks over C
NFT = FF // P      # 32 f-tiles
NTT = T // P       # 16 t-tiles
NTB = T // 512     # 4 t-blocks (attention bands)

F16 = mybir.dt.float16
F32 = mybir.dt.float32
MASK_NEG = -30000.0

_CACHED_NC = None
LAST_RESULTS = None


def _build_module():
    nc = bacc.Bacc("TRN2", target_bir_lowering=False, debug=False,
                   num_devices=NCORES)

    x_in = nc.dram_tensor("x", [T, C], F32, kind="ExternalInput").ap()
    xres_in = nc.dram_tensor("xres", [TLOC, C], F32, kind="ExternalInput").ap()
    wq_in = nc.dram_tensor("wq", [P, KO, NHL * HD], F16, kind="ExternalInput").ap()
    wk_in = nc.dram_tensor("wk", [P, KO, NHL * HD], F16, kind="ExternalInput").ap()
    wv_in = nc.dram_tensor("wv", [P, KO, NHL * HD], F16, kind="ExternalInput").ap()
    bqk_in = nc.dram_tensor("bqk", [P, 4], F32, kind="ExternalInput").ap()
    bv_in = nc.dram_tensor("bv", [1, NHL * HD], F32, kind="ExternalInput").ap()
    wo_in = nc.dram_tensor("wo", [P, 2, C], F16, kind="ExternalInput").ap()
    w1_in = nc.dram_tensor("w1", [P, NFT, KO, P], F16, kind="ExternalInput").ap()
    b1_in = nc.dram_tensor("b1p", [P, NFT], F32, kind="ExternalInput").ap()
    w2_in = nc.dram_tensor("w2", [P, NFT, C], F16, kind="ExternalInput").ap()
    b2_in = nc.dram_tensor("b2", [1, C], F32, kind="ExternalInput").ap()
    y_out = nc.dram_tensor("y", [TLOC, C], F32, kind="ExternalOutput").ap()

    with tile.TileContext(nc) as tc:
        _emit(nc, tc, x_in, xres_in, wq_in, wk_in, wv_in, bqk_in, bv_in,
              wo_in, w1_in, b1_in, w2_in, b2_in, y_out)
    nc.compile()
    return nc


def _layernorm_z(nc, stats, xt, z_out):
    """z = (x - mean(x)) / (unbiased_std(x) + EPS), rows on partitions.

    xt: [P, C] fp32 SBUF tile (an AP with free size C); z_out: [P, C] F16."""
    s6 = stats.tile([P, 2, 6], F32, tag="bn6")
    nc.vector.bn_stats(s6[:, 0, :], xt[:, 0:C // 2])
    nc.vector.bn_stats(s6[:, 1, :], xt[:, C // 2:C])
    mv = stats.tile([P, 2], F32, tag="bnmv")
    nc.vector.bn_aggr(mv[:], s6[:])
    std = stats.tile([P, 1], F32, tag="bnstd")
    # unbiased std = sqrt(var_pop*C/(C-1))
    nc.scalar.activation(std[:], mv[:, 1:2], mybir.ActivationFunctionType.Sqrt,
                         scale=float(C) / float(C - 1))
    rstd = stats.tile([P, 1], F32, tag="bnrstd")
    nc.vector.reciprocal(rstd[:], std[:])
    return nc.vector.tensor_scalar(z_out[:], xt[:], mv[:, 0:1], rstd[:],
                                   mybir.AluOpType.subtract,
                                   mybir.AluOpType.mult)


def _emit(nc, tc, x_in, xres_in, wq_in, wk_in, wv_in, bqk_in, bv_in,
          wo_in, w1_in, b1_in, w2_in, b2_in, y_out):
    ctx = contextlib.ExitStack()
    # persistent pools (whole kernel)
    fp = ctx.enter_context(tc.tile_pool(name="fixed", bufs=1))
    stats = ctx.enter_context(tc.tile_pool(name="stats", bufs=6))
    ztiles = ctx.enter_context(tc.tile_pool(name="ztiles", bufs=3))
    dram = ctx.enter_context(tc.tile_pool(name="dram", bufs=1, space="DRAM"))

    # --- persistent constants -----------------------------------------------
    ones1h = fp.tile([1, P], F16)
    nc.vector.memset(ones1h[:], 1.0)
    b1p = fp.tile([P, NFT], F32)
    nc.scalar.dma_start(b1p[:], b1_in[:])
    b2 = fp.tile([1, C], F32)
    nc.scalar.dma_start(b2[:], b2_in[:])
    b2h = fp.tile([1, C], F16)
    nc.vector.tensor_copy(b2h[:], b2[:])

    rs_ins = [dram.tile([512, C], F16, name=f"rsin{j}") for j in range(NTB)]
    rs_outs = [dram.tile([P, C], F16, name=f"rsout{j}") for j in range(NTB)]

    # FFN W1 fully resident in SBUF (prefetched in chunks on the sync queue
    # during attention; ring-semaphore reuse throttles it behind the x loads).
    w1sb = ctx.enter_context(
        tc.tile_pool(name="w1sb", bufs=1, side="right")).tile(
        [P, NFT, KO, P], F16)

    # attention-scope pools: released after attention
    actx = contextlib.ExitStack()
    fpa = actx.enter_context(tc.tile_pool(name="fixeda", bufs=1))
    abig = actx.enter_context(tc.tile_pool(name="abig", bufs=1))

    zero512 = fpa.tile([P, 512], F16)
    nc.vector.memset(zero512[:], 0.0)
    masks = fpa.tile([P, 4, 512], F16)
    for k in range(4):
        # keep score where (t_rel - s_rel - 128k) >= 0 else MASK_NEG
        nc.gpsimd.affine_select(
            out=masks[:, k, :], in_=zero512[:],
            compare_op=mybir.AluOpType.is_ge, fill=MASK_NEG,
            base=-128 * k, channel_multiplier=-1, pattern=[[1, 512]],
        )
    wo = fpa.tile([P, 2, C], F16)

    qT2 = abig.tile([P, 2, T], F16)
    kT2 = abig.tile([P, 2, T], F16)
    v_sb = abig.tile([P, NTT, NHL, HD + 1], F16)
    ones_c = fpa.tile([P, 1], F16)
    nc.vector.memset(ones_c[:], 1.0)
    nc.vector.tensor_copy(
        v_sb[:, :, :, HD:HD + 1],
        ones_c[:, :, None, None].to_broadcast((P, NTT, NHL, 1)))

    # persistent across attention->FFN
    x2 = ctx.enter_context(tc.tile_pool(name="x2p", bufs=1, side="right")).tile(
        [P, TLOC // P, C], F32)
    xn2T = ctx.enter_context(
        tc.tile_pool(name="xn2Tp", bufs=1, side="right")).tile(
        [P, KO, TLOC], F16)

    # --- phase pools (LIFO: p12 closes after last QKV, attnps before FFN) ----
    pctx = contextlib.ExitStack()
    ptp = pctx.enter_context(tc.tile_pool(name="ptp", bufs=3))
    mskp = pctx.enter_context(tc.tile_pool(name="mskp", bufs=2))
    rzp = pctx.enter_context(tc.tile_pool(name="rzp", bufs=4))
    pairp = pctx.enter_context(tc.tile_pool(name="pairp", bufs=4))
    bandp = pctx.enter_context(tc.tile_pool(name="bandp", bufs=2))
    rstage = pctx.enter_context(tc.tile_pool(name="rstage", bufs=2))
    p6 = pctx.enter_context(tc.tile_pool(name="p6", bufs=1))
    # PSUM: sc tag [P,1024]x2 = 4 banks (scores hh0+hh1 live concurrently, and
    # the v projections); big tag [P,512]x2 = 2 banks (QK psum + Wo out);
    # cq tag x2 = 2 banks.  Total 8.
    attnps1 = contextlib.ExitStack()
    psbig = attnps1.enter_context(
        tc.tile_pool(name="psbig", bufs=2, space="PSUM"))
    attnps2 = contextlib.ExitStack()
    pssc = attnps2.enter_context(tc.tile_pool(name="pssc", bufs=2, space="PSUM"))
    psctxq = attnps2.enter_context(
        tc.tile_pool(name="psctxq", bufs=2, space="PSUM"))

    p12 = contextlib.ExitStack()
    qkvw = p12.enter_context(tc.tile_pool(name="qkvw", bufs=1))
    bqk = qkvw.tile([P, 4], F32)
    bv = qkvw.tile([1, NHL * HD], F32)
    bv_h = qkvw.tile([1, NHL * HD], F16)
    wq = qkvw.tile([P, KO, NHL * HD], F16)
    wk = qkvw.tile([P, KO, NHL * HD], F16)
    wv = qkvw.tile([P, KO, NHL * HD], F16)

    def emit_qkv_loads():
        # scalar HWDGE queue: runs concurrently with the x loads on sync
        nc.scalar.dma_start(bqk[:], bqk_in[:])
        nc.scalar.dma_start(bv[:], bv_in[:])
        nc.vector.tensor_copy(bv_h[:], bv[:])
        nc.scalar.dma_start(wq[:], wq_in[:])
        nc.scalar.dma_start(wk[:], wk_in[:])
        nc.scalar.dma_start(wv[:], wv_in[:])
    xnTp = p12.enter_context(tc.tile_pool(name="xnTp", bufs=2))
    xtiles = p12.enter_context(tc.tile_pool(name="xtiles", bufs=3))

    xnTs = {}
    ctxbs = {}
    xload = [None]

    def emit_p1(tb):
        """x load + LN1 + XBAR transpose for 4 tiles of band tb.

        All 4 x loads are emitted before any XBAR so the sync queue's
        in-order dispatch never parks a load behind an XBAR's z-wait.
        For bands 0-1 (latency-critical, before the scalar engine starts
        the softmax exps) each z transpose is split in half across the two
        HWDGE queues; bands 2-3 go as single dispatches on sync so the
        scalar queue stays free for exps."""
        xnT = xnTp.tile([P, KO, 512], F16, tag="xnT")
        xnTs[tb] = xnT
        zs = []
        for tt4 in range(4):
            tt = 4 * tb + tt4
            xt = xtiles.tile([P, C], F32, tag="x")
            xload[0] = nc.sync.dma_start(xt[:], x_in[tt * P:(tt + 1) * P, :])
            z = ztiles.tile([P, C], F16, tag="z")
            _layernorm_z(nc, stats, xt, z)
            zs.append(z)
        for tt4 in range(4):
            if tb < 2:
                nc.sync.dma_start_transpose(
                    xnT[:, 0:KO // 2, tt4 * P:(tt4 + 1) * P],
                    zs[tt4][:, 0:C // 2])
                nc.scalar.dma_start_transpose(
                    xnT[:, KO // 2:KO, tt4 * P:(tt4 + 1) * P],
                    zs[tt4][:, C // 2:C])
            else:
                nc.sync.dma_start_transpose(
                    xnT[:, :, tt4 * P:(tt4 + 1) * P], zs[tt4][:])

    def emit_w1_prefetch(lo, hi):
        # W1 prefetch chunks on the sync HWDGE queue; the hardware ring
        # semaphores throttle them behind earlier (x) loads automatically.
        for cchunk in range(lo, hi):
            nc.sync.dma_start(
                w1sb[:, 2 * cchunk:2 * cchunk + 2, :, :],
                w1_in[:, 2 * cchunk:2 * cchunk + 2, :, :])

    def emit_p2(tb):
        """QKV projections for band tb from xnT."""
        xnT = xnTs.pop(tb)
        for pp in range(2):
            for dst, w, bcol in ((qT2, wq, pp), (kT2, wk, 2 + pp)):
                ps = psbig.tile([P, 512], F32, tag="big")
                for ko in range(KO):
                    nc.tensor.matmul(
                        ps[:], w[:, ko, pp * P:(pp + 1) * P],
                        xnT[:, ko, :],
                        start=(ko == 0), stop=(ko == KO - 1))
                nc.vector.tensor_scalar_add(
                    dst[:, pp, tb * 512:(tb + 1) * 512], ps[:],
                    bqk[:, bcol:bcol + 1])
        for tt4 in range(4):
            tt = 4 * tb + tt4
            psf = pssc.tile([P, 1024], F32, tag="sc")
            ps = psf[:, 0:NHL * HD]
            nc.tensor.matmul(ps, ones1h[0:1, :], bv_h[0:1, :],
                             start=True, stop=False, skip_group_check=True)
            for ko in range(KO):
                nc.tensor.matmul(
                    ps, xnT[:, ko, tt4 * P:(tt4 + 1) * P], wv[:, ko, :],
                    start=False, stop=(ko == KO - 1),
                    skip_group_check=True)
            nc.vector.tensor_copy(
                v_sb[:, tt, :, 0:HD],
                ps.rearrange("p (h d) -> p h d", h=NHL))

    def emit_band(j, mid=None):
        """Attention band j (512 query rows), 4 local heads, then Wo.

        The two heads of a pair (hh=0 on partitions 0-63, hh=1 on 64-127)
        have their K=64 score matmuls emitted adjacently: the PE runs them
        concurrently on row-groups 0-63 / 64-127 (auto tile_position from
        base_partition), into different PSUM banks.

        ctx accumulates as [tokens, HD+1] per 128-token chunk, so softmax Z
        lands per-partition: the 1/Z normalize is a [P,1] reciprocal plus a
        [P,64] scale.  Normalized ctx pairs (2 heads = 128 cols) go back to
        the [head-cols, tokens] layout via XBAR transposes, alternating
        between the two HWDGE queues."""
        ctxb = bandp.tile([P, 2, 512], F16, tag="ctxb")
        assert j not in ctxbs
        for pp in range(2):
            pairT = pairp.tile([P, 4, P], F16, tag="pair")
            cqs = [psctxq.tile([P, 4, HD + 1], F32, tag="cq", name=f"cq{hh}")
                   for hh in range(2)]
            # paired off-diagonal blocks; hh=0/1 score MMs interleaved
            for pi in range(2 * j):
                sps2 = [pssc.tile([P, 1024], F32, tag="sc", name=f"sps{hh}")
                        for hh in range(2)]
                for half in range(2):
                    i = 2 * pi + half
                    for hh in range(2):
                        poff = 64 * hh
                        nc.tensor.matmul(
                            sps2[hh][:, half * 512:(half + 1) * 512],
                            kT2[poff:poff + HD, pp, i * P:(i + 1) * P],
                            qT2[poff:poff + HD, pp, j * 512:(j + 1) * 512],
                            start=True, stop=True)
                for hh in range(2):
                    pT = ptp.tile([P, 1024], F16, tag="pT")
                    nc.scalar.activation(pT[:], sps2[hh][:],
                                         mybir.ActivationFunctionType.Exp,
                                         scale=0.125)
                    for half in range(2):
                        i = 2 * pi + half
                        for tcc in range(4):
                            nc.tensor.matmul(
                                cqs[hh][:, tcc, :],
                                pT[:, half * 512 + tcc * P:
                                   half * 512 + (tcc + 1) * P],
                                v_sb[:, i, 2 * pp + hh, :],
                                start=(i == 0 and tcc == 0), stop=False,
                                skip_group_check=True)
            # diagonal blocks, live-column sliced; hh=1 uses cols 512:512+live
            # of the same [P,1024] psum tile (a different PSUM bank).
            for k in range(4):
                i = 4 * j + k
                lo = 128 * k
                live = 512 - lo
                sps = pssc.tile([P, 1024], F32, tag="sc")
                for hh in range(2):
                    poff = 64 * hh
                    nc.tensor.matmul(
                        sps[:, 512 * hh:512 * hh + live],
                        kT2[poff:poff + HD, pp, i * P:(i + 1) * P],
                        qT2[poff:poff + HD, pp,
                            j * 512 + lo:(j + 1) * 512],
                        start=True, stop=True, skip_group_check=True)
                for hh in range(2):
                    ms = mskp.tile([P, 512], F16, tag="ms")
                    nc.vector.scalar_tensor_tensor(
                        ms[:, 0:live], sps[:, 512 * hh:512 * hh + live],
                        0.125, masks[:, k, lo:512],
                        mybir.AluOpType.mult, mybir.AluOpType.add)
                    pT = ptp.tile([P, 1024], F16, tag="pT")
                    nc.scalar.activation(pT[:, 0:live], ms[:, 0:live],
                                         mybir.ActivationFunctionType.Exp)
                    for tcc in range(k, 4):
                        nc.tensor.matmul(
                            cqs[hh][:, tcc, :],
                            pT[:, (tcc - k) * P:(tcc - k + 1) * P],
                            v_sb[:, i, 2 * pp + hh, :],
                            start=(i == 0 and tcc == 0),
                            stop=(k == 3 and tcc == 3),
                            skip_group_check=True)
            # normalize per token chunk: [P,1] reciprocal + [P,64] scale
            for hh in range(2):
                poff = 64 * hh
                for tcc in range(4):
                    rzq = rzp.tile([P, 1], F32, tag="rzq")
                    nc.vector.reciprocal(rzq[:], cqs[hh][:, tcc, HD:HD + 1])
                    nc.vector.tensor_scalar_mul(
                        pairT[:, tcc, poff:poff + HD],
                        cqs[hh][:, tcc, 0:HD], rzq[:])
            # one merged XBAR dispatch: ctxb[hd, tc*128+tk] = pairT[tk, tc, hd]
            nc.sync.dma_start_transpose(
                ctxb[:, pp, :].rearrange("p (b c) -> p b c", c=P),
                pairT[:])
            if mid is not None and pp == 0:
                mid()
        ctxbs[j] = ctxb

    def emit_band_wo(j):
        # Wo partials for this band -> rs_ins[j]; stores on the sync queue
        ctxb = ctxbs.pop(j)
        for tt4 in range(4):
            stg = rstage.tile([P, C], F16, tag="stg")
            for cb in range(2):
                ops_ = psbig.tile([P, 512], F32, tag="big")
                for ch in range(2):
                    nc.tensor.matmul(
                        ops_[:],
                        ctxb[:, ch, tt4 * P:(tt4 + 1) * P],
                        wo[:, ch, cb * 512:(cb + 1) * 512],
                        start=(ch == 0), stop=(ch == 1))
                nc.vector.tensor_copy(stg[:, cb * 512:(cb + 1) * 512],
                                      ops_[:])
            nc.sync.dma_start(
                rs_ins[j][tt4 * P:(tt4 + 1) * P, :], stg[:])

    def emit_rs(j):
        nc.gpsimd.collective_compute(
            "ReduceScatter", mybir.AluOpType.add,
            replica_groups=[[0, 1, 2, 3], [4, 5, 6, 7]],
            ins=[rs_ins[j].opt()], outs=[rs_outs[j].opt()],
        )

    def emit_p6(j):
        """x2 row block j = rs_out[j] + (xres+bo); LN2; transpose to xn2T."""
        rst = p6.tile([P, C], F16, tag="rst")
        nc.sync.dma_start(rst[:], rs_outs[j][:])
        xrt = p6.tile([P, C], F32, tag="xrt")
        nc.scalar.dma_start(xrt[:], xres_in[j * P:(j + 1) * P, :])
        nc.vector.tensor_tensor(x2[:, j, :], rst[:], xrt[:],
                                mybir.AluOpType.add)
        z2 = ztiles.tile([P, C], F16, tag="z")
        _layernorm_z(nc, stats, x2[:, j, :], z2)
        # both halves on sync: scalar must stay free for exps/relus
        nc.sync.dma_start_transpose(
            xn2T[:, 0:KO // 2, j * P:(j + 1) * P], z2[:, 0:C // 2])
        nc.sync.dma_start_transpose(
            xn2T[:, KO // 2:KO, j * P:(j + 1) * P], z2[:, C // 2:C])

    # --- P1/P2 + attention, interleaved -------------------------------------
    emit_qkv_loads()
    emit_p1(0)
    nc.scalar.dma_start(wo[:], wo_in[:])
    emit_p1(1)
    emit_p2(0)
    emit_p2(1)
    emit_p1(2)
    emit_band(0)
    emit_band_wo(0)
    emit_rs(0)
    emit_w1_prefetch(0, 6)
    emit_p2(2)
    emit_p1(3)
    emit_band(1)
    emit_band_wo(1)
    emit_rs(1)
    emit_w1_prefetch(6, 12)
    emit_p2(3)
    p12.close()
    # p6(0)/p6(1) are emitted mid-band, at points where their RS is
    # certainly complete, so their vector/scalar ops never head-of-line
    # block the attention work queued behind them.
    emit_band(2, mid=lambda: emit_p6(0))
    emit_w1_prefetch(12, 16)
    emit_band_wo(2)
    emit_rs(2)
    emit_band(3, mid=lambda: emit_p6(1))
    emit_band_wo(3)
    emit_rs(3)
    attnps2.close()

    # --- FFN -----------------------------------------------------------------
    with tc.tile_pool(name="w2p", bufs=12) as w2p, \
         tc.tile_pool(name="rp", bufs=2) as rp, \
         tc.tile_pool(name="psh1", bufs=2, space="PSUM") as psh1, \
         tc.tile_pool(name="psh2", bufs=4, space="PSUM") as psh2, \
         tc.tile_pool(name="yp", bufs=2) as yp:

        def ffn_half(th, mid_cbs=()):
            mid_cbs = dict(mid_cbs)
            h2ps = []
            for tt2 in range(2):
                for cb in range(2):
                    hp = psh2.tile([P, 512], F32, tag="h2")
                    nc.tensor.matmul(hp[:], ones1h[0:1, :],
                                     b2h[0:1, cb * 512:(cb + 1) * 512],
                                     start=True, stop=False,
                                     skip_group_check=True)
                    h2ps.append(hp)
            for ft in range(NFT):
                if ft in mid_cbs:
                    tc.no_sync_barrier()
                    mid_cbs[ft]()
                w2t = w2p.tile([P, C], F16, tag="w2")
                nc.sync.dma_start(w2t[:], w2_in[:, ft, :])
                h1 = psh1.tile([P, 256], F32, tag="h1")
                for ko in range(KO):
                    nc.tensor.matmul(h1[:], w1sb[:, ft, ko, :],
                                     xn2T[:, ko, th * 256:(th + 1) * 256],
                                     start=(ko == 0), stop=(ko == KO - 1))
                rT = rp.tile([P, 256], F16, tag="rT")
                # relu+bias on the scalar engine (idle during FFN)
                nc.scalar.activation(rT[:], h1[:],
                                     mybir.ActivationFunctionType.Relu,
                                     bias=b1p[:, ft:ft + 1])
                for tt2 in range(2):
                    for cb in range(2):
                        nc.tensor.matmul(
                            h2ps[2 * tt2 + cb][:],
                            rT[:, tt2 * P:(tt2 + 1) * P],
                            w2t[:, cb * 512:(cb + 1) * 512],
                            start=False, stop=(ft == NFT - 1),
                            skip_group_check=True)
            for tt2 in range(2):
                gt = 2 * th + tt2
                for cb in range(2):
                    yt = yp.tile([P, 512], F32, tag="y")
                    nc.vector.scalar_tensor_tensor(
                        yt[:], h2ps[2 * tt2 + cb][:], 1.0,
                        x2[:, gt, cb * 512:(cb + 1) * 512],
                        mybir.AluOpType.mult, mybir.AluOpType.add)
                    nc.sync.dma_start(
                        y_out[gt * P:(gt + 1) * P,
                              cb * 512:(cb + 1) * 512],
                        yt[:])

        ffn_half(0, mid_cbs={2: lambda: emit_p6(2), 22: lambda: emit_p6(3)})
        ffn_half(1)

    attnps1.close()
    pctx.close()
    actx.close()
    ctx.close()


def _prep_inputs(x, Wq, Wk, Wv, Wo, bo, W1, b1, W2, b2, g1, be1, g2, be2):
    """Host-side sharding + layout packing. Returns list of 8 in_maps."""
    f32 = np.float32
    f16 = np.float16
    x = np.asarray(x, f32)
    Wq, Wk, Wv = (np.asarray(a, f32) for a in (Wq, Wk, Wv))
    Wo, bo = np.asarray(Wo, f32), np.asarray(bo, f32)
    W1, b1, W2, b2 = (np.asarray(a, f32) for a in (W1, b1, W2, b2))
    g1, be1, g2, be2 = (np.asarray(a, np.float64) for a in (g1, be1, g2, be2))

    def pack_qkv(W):  # [NHL, C, HD] g-folded -> [P, KO, NHL*HD] fp16
        Wl = (g1[None, :, None] * W.astype(np.float64)).astype(f32)
        flat = Wl.transpose(1, 0, 2).reshape(C, NHL * HD)   # [c, col]
        return np.ascontiguousarray(flat.reshape(KO, P, NHL * HD)
                                    .transpose(1, 0, 2)).astype(f16)

    # W1 folded with g2: [C, FF] -> [P, NFT, KO, P]
    W1f = (g2[:, None] * W1.astype(np.float64)).astype(f32)
    w1_arr = np.ascontiguousarray(
        W1f.reshape(KO, P, NFT, P).transpose(1, 2, 0, 3)).astype(f16)
    b1p = (b1.astype(np.float64) + be2 @ W1.astype(np.float64)).astype(f32)
    b1_arr = np.ascontiguousarray(b1p.reshape(NFT, P).T)
    w2_arr = np.ascontiguousarray(
        W2.reshape(NFT, P, C).transpose(1, 0, 2)).astype(f16)
    b2_arr = b2.reshape(1, C)

    in_maps = []
    for core in range(NCORES):
        b, r = divmod(core, TPG)
        hsel = slice(NHL * r, NHL * (r + 1))
        wq_arr = pack_qkv(Wq[hsel])
        wk_arr = pack_qkv(Wk[hsel])
        wv_arr = pack_qkv(Wv[hsel])
        # be1-induced biases (exact): col order = head-major within 256
        bq = (be1 @ Wq[hsel].astype(np.float64).transpose(1, 0, 2)
              .reshape(C, NHL * HD)).astype(f32)
        bk = (be1 @ Wk[hsel].astype(np.float64).transpose(1, 0, 2)
              .reshape(C, NHL * HD)).astype(f32)
        bvv = (be1 @ Wv[hsel].astype(np.float64).transpose(1, 0, 2)
               .reshape(C, NHL * HD)).astype(f32)
        bqk_arr = np.stack([bq[0:P], bq[P:2 * P], bk[0:P], bk[P:2 * P]],
                           axis=1).astype(f32)
        wo_arr = np.ascontiguousarray(
            Wo[NHL * HD * r: NHL * HD * (r + 1)].reshape(2, P, C)
            .transpose(1, 0, 2)).astype(f16)
        # per-band RS: core r owns rows 512j + 128r + [0,128) for band j
        li = np.arange(TLOC)
        lidx = 512 * (li // P) + P * r + (li % P)
        in_maps.append({
            "x": x[b],
            "xres": np.ascontiguousarray(x[b, lidx] + bo[None, :]),
            "wq": wq_arr, "wk": wk_arr, "wv": wv_arr,
            "bqk": bqk_arr, "bv": bvv.reshape(1, NHL * HD),
            "wo": wo_arr,
            "w1": w1_arr, "b1p": b1_arr, "w2": w2_arr, "b2": b2_arr,
        })
    return in_maps


def kernel(**inputs):
    global _CACHED_NC, LAST_RESULTS
    if _CACHED_NC is None:
        _CACHED_NC = _build_module()
    in_maps = _prep_inputs(**inputs)
    res = bass_utils.run_bass_kernel_spmd(
        _CACHED_NC, in_maps, core_ids=list(range(NCORES)))
    LAST_RESULTS = res
    y = np.empty((B, T, C), np.float32)
    li = np.arange(TLOC)
    lidx0 = 512 * (li // P) + (li % P)
    for core in range(NCORES):
        b, r = divmod(core, TPG)
        y[b, lidx0 + P * r] = res.results[core]["y"]
    return y


# revision 23
# speedup vs baseline: 1.4211x; 1.4211x over previous
"""Trainium2 Bass kernel for a dense transformer block (B=2, T=2048, C=1024,
NH=16, HD=64, FF=4x), distributed over 8 NeuronCores.

Sharding: data-parallel over batch (2 groups of 4 cores) x tensor-parallel over
heads within a group (4 heads/core), with sequence-parallel FFN: attention
output partials are ReduceScattered over T inside each group as 4 per-band
collectives (512 rows each), then each core runs LN2+FFN on its own 512 rows.

All matmul operands are fp16 (error gate is 2e-2; fp16 keeps ~3e-4).
LayerNorm stats, PSUM accumulation and the residual stream stay fp32.
Transposes run on the DMA XBAR, split across the two HWDGE queues (sync +
scalar) so two halves proceed in parallel.  The gpsimd queue is reserved for
the collectives so their doorbells never sit behind blocked DMA dispatches.
Score matmuls (K=64) for the two heads of a pair are emitted adjacently so
they run concurrently on PE row-groups 0-63 / 64-127.
LN gains are folded into the weights host-side (exact algebra):
  xn = g*z + be  (z = (x-mean)/std)  =>  xn @ W = z @ (g*W) + be @ W
"""

import contextlib
import os
import sys
import types

import numpy as np

# --- NTFF profile hook shim (tracing support; harmless when unused) ---------
def _install_ntff_hook_shim():
    if "antenv.axon_hooks" in sys.modules:
        return
    try:
        import antenv
        import trn_agent_boot.trn_boot as tb

        mod = types.ModuleType("antenv.axon_hooks")
        holder = [None]
        mod.set_axon_ntff_profile_hook = lambda h: holder.__setitem__(0, h)
        mod.get_axon_ntff_profile_hook = lambda: holder[0]
        sys.modules["antenv.axon_hooks"] = mod
        antenv.axon_hooks = mod
        if os.path.exists("/opt/axon/libaxon_pjrt.so"):
            mod.set_axon_ntff_profile_hook(
                tb._ntff_profile_via_ctypes("/opt/axon/libaxon_pjrt.so")
            )
    except Exception:
        pass


_install_ntff_hook_shim()

import concourse.bass as bass
import concourse.mybir as mybir
import concourse.tile as tile
from concourse.tile_rust import add_dep_helper
from concourse import bacc
from concourse import bass_utils

# Problem shape (hardcoded per contest rules).
B, T, C, NH, HD = 2, 2048, 1024, 16, 64
FF = 4 * C  # 4096
EPS = 1e-6
P = 128
NCORES = 8
TPG = 4            # cores per batch group
NHL = NH // TPG    # local heads per core = 4
TLOC = T // TPG    # rows per core after ReduceScatter = 512
KO = C // P        # 8 contraction chunks over C
NFT = FF // P      # 32 f-tiles
NTT = T // P       # 16 t-tiles
NTB = T // 512     # 4 t-blocks (attention bands)

F16 = mybir.dt.float16
F32 = mybir.dt.float32
MASK_NEG = -30000.0

_CACHED_NC = None
LAST_RESULTS = None


def _build_module():
    nc = bacc.Bacc("TRN2", target_bir_lowering=False, debug=False,
                   num_devices=NCORES)

    x_in = nc.dram_tensor("x", [T, C], F32, kind="ExternalInput").ap()
    xres_in = nc.dram_tensor("xres", [TLOC, C], F32, kind="ExternalInput").ap()
    wq_in = nc.dram_tensor("wq", [P, KO, NHL * HD], F16, kind="ExternalInput").ap()
    wk_in = nc.dram_tensor("wk", [P, KO, NHL * HD], F16, kind="ExternalInput").ap()
    wv_in = nc.dram_tensor("wv", [P, KO, NHL * HD], F16, kind="ExternalInput").ap()
    bqk_in = nc.dram_tensor("bqk", [P, 4], F32, kind="ExternalInput").ap()
    bv_in = nc.dram_tensor("bv", [1, NHL * HD], F32, kind="ExternalInput").ap()
    wo_in = nc.dram_tensor("wo", [P, 2, C], F16, kind="ExternalInput").ap()
    w1_in = nc.dram_tensor("w1", [P, NFT, KO, P], F16, kind="ExternalInput").ap()
    b1_in = nc.dram_tensor("b1p", [P, NFT], F32, kind="ExternalInput").ap()
    w2_in = nc.dram_tensor("w2", [P, NFT, C], F16, kind="ExternalInput").ap()
    b2_in = nc.dram_tensor("b2", [1, C], F32, kind="ExternalInput").ap()
    y_out = nc.dram_tensor("y", [TLOC, C], F32, kind="ExternalOutput").ap()

    with tile.TileContext(nc) as tc:
        _emit(nc, tc, x_in, xres_in, wq_in, wk_in, wv_in, bqk_in, bv_in,
              wo_in, w1_in, b1_in, w2_in, b2_in, y_out)
    nc.compile()
    return nc


def _layernorm_z(nc, stats, xt, z_out):
    """z = (x - mean(x)) / (unbiased_std(x) + EPS), rows on partitions.

    xt: [P, C] fp32 SBUF tile (an AP with free size C); z_out: [P, C] F16."""
    s6 = stats.tile([P, 2, 6], F32, tag="bn6")
    nc.vector.bn_stats(s6[:, 0, :], xt[:, 0:C // 2])
    nc.vector.bn_stats(s6[:, 1, :], xt[:, C // 2:C])
    mv = stats.tile([P, 2], F32, tag="bnmv")
    nc.vector.bn_aggr(mv[:], s6[:])
    std = stats.tile([P, 1], F32, tag="bnstd")
    # unbiased std = sqrt(var_pop*C/(C-1))
    nc.scalar.activation(std[:], mv[:, 1:2], mybir.ActivationFunctionType.Sqrt,
                         scale=float(C) / float(C - 1))
    rstd = stats.tile([P, 1], F32, tag="bnrstd")
    nc.vector.reciprocal(rstd[:], std[:])
    return nc.vector.tensor_scalar(z_out[:], xt[:], mv[:, 0:1], rstd[:],
                                   mybir.AluOpType.subtract,
                                   mybir.AluOpType.mult)


def _emit(nc, tc, x_in, xres_in, wq_in, wk_in, wv_in, bqk_in, bv_in,
          wo_in, w1_in, b1_in, w2_in, b2_in, y_out):
    ctx = contextlib.ExitStack()
    # persistent pools (whole kernel)
    fp = ctx.enter_context(tc.tile_pool(name="fixed", bufs=1))
    stats = ctx.enter_context(tc.tile_pool(name="stats", bufs=6))
    ztiles = ctx.enter_context(tc.tile_pool(name="ztiles", bufs=3))
    dram = ctx.enter_context(tc.tile_pool(name="dram", bufs=1, space="DRAM"))

    # --- persistent constants -----------------------------------------------
    ones1h = fp.tile([1, P], F16)
    nc.vector.memset(ones1h[:], 1.0)
    b1p = fp.tile([P, NFT], F32)
    nc.scalar.dma_start(b1p[:], b1_in[:])
    b2 = fp.tile([1, C], F32)
    nc.scalar.dma_start(b2[:], b2_in[:])
    b2h = fp.tile([1, C], F16)
    nc.vector.tensor_copy(b2h[:], b2[:])

    rs_ins = [dram.tile([512, C], F16, name=f"rsin{j}") for j in range(NTB)]
    rs_outs = [dram.tile([P, C], F16, name=f"rsout{j}") for j in range(NTB)]
    # warmup collective buffers: a tiny ReduceScatter fired at kernel start
    # absorbs cross-core launch skew + CC warmup off the critical path, so
    # the real per-band collectives run at steady-state (~15us/MB).
    wu_in = dram.tile([4, 64], F16, name="wuin")
    wu_out = dram.tile([1, 64], F16, name="wuout")

    # FFN W1 fully resident in SBUF (prefetched in chunks on the sync queue
    # during attention; ring-semaphore reuse throttles it behind the x loads).
    w1sb = ctx.enter_context(
        tc.tile_pool(name="w1sb", bufs=1, side="right")).tile(
        [P, NFT, KO, P], F16)

    # attention-scope pools: released after attention
    actx = contextlib.ExitStack()
    fpa = actx.enter_context(tc.tile_pool(name="fixeda", bufs=1))
    abig = actx.enter_context(tc.tile_pool(name="abig", bufs=1))

    zero512 = fpa.tile([P, 512], F16)
    nc.vector.memset(zero512[:], 0.0)
    masks = fpa.tile([P, 4, 512], F16)
    for k in range(4):
        # keep score where (t_rel - s_rel - 128k) >= 0 else MASK_NEG
        nc.gpsimd.affine_select(
            out=masks[:, k, :], in_=zero512[:],
            compare_op=mybir.AluOpType.is_ge, fill=MASK_NEG,
            base=-128 * k, channel_multiplier=-1, pattern=[[1, 512]],
        )
    onespp = fpa.tile([P, P], F16)
    nc.vector.memset(onespp[:], 1.0)
    ident = fpa.tile([P, P], F16)
    nc.gpsimd.affine_select(
        out=ident[:], in_=onespp[:],
        compare_op=mybir.AluOpType.is_equal, fill=0.0,
        base=0, channel_multiplier=-1, pattern=[[1, P]],
    )
    # fire the warmup collective as early as possible
    wus = fpa.tile([4, 64], F16)
    nc.vector.memset(wus[:], 0.0)
    nc.gpsimd.dma_start(wu_in[:], wus[:])
    nc.gpsimd.collective_compute(
        "ReduceScatter", mybir.AluOpType.add,
        replica_groups=[[0, 1, 2, 3], [4, 5, 6, 7]],
        ins=[wu_in.opt()], outs=[wu_out.opt()],
    )
    wo = fpa.tile([P, 2, C], F16)

    qT2 = abig.tile([P, 2, T], F16)
    kT2 = abig.tile([P, 2, T], F16)
    v_sb = abig.tile([P, NTT, NHL, HD + 1], F16)
    ones_c = fpa.tile([P, 1], F16)
    nc.vector.memset(ones_c[:], 1.0)
    nc.vector.tensor_copy(
        v_sb[:, :, :, HD:HD + 1],
        ones_c[:, :, None, None].to_broadcast((P, NTT, NHL, 1)))

    # persistent across attention->FFN
    x2 = ctx.enter_context(tc.tile_pool(name="x2p", bufs=1, side="right")).tile(
        [P, TLOC // P, C], F32)
    xn2T = ctx.enter_context(
        tc.tile_pool(name="xn2Tp", bufs=1, side="right")).tile(
        [P, KO, TLOC], F16)

    # --- phase pools (LIFO: p12 closes after last QKV, attnps before FFN) ----
    pctx = contextlib.ExitStack()
    ptp = pctx.enter_context(tc.tile_pool(name="ptp", bufs=3))
    mskp = pctx.enter_context(tc.tile_pool(name="mskp", bufs=2))
    rzp = pctx.enter_context(tc.tile_pool(name="rzp", bufs=4))
    pairp = pctx.enter_context(tc.tile_pool(name="pairp", bufs=4))
    bandp = pctx.enter_context(tc.tile_pool(name="bandp", bufs=2))
    rstage = pctx.enter_context(tc.tile_pool(name="rstage", bufs=2))
    p6 = pctx.enter_context(tc.tile_pool(name="p6", bufs=1))
    # PSUM: sc tag [P,1024]x2 = 4 banks (scores hh0+hh1 live concurrently, and
    # the v projections); big tag [P,512]x2 = 2 banks (QK psum + Wo out);
    # cq tag x2 = 2 banks.  Total 8.
    attnps1 = contextlib.ExitStack()
    psbig = attnps1.enter_context(
        tc.tile_pool(name="psbig", bufs=2, space="PSUM"))
    attnps2 = contextlib.ExitStack()
    pssc = attnps2.enter_context(tc.tile_pool(name="pssc", bufs=2, space="PSUM"))
    psctxq = attnps2.enter_context(
        tc.tile_pool(name="psctxq", bufs=2, space="PSUM"))

    p12 = contextlib.ExitStack()
    qkvw = p12.enter_context(tc.tile_pool(name="qkvw", bufs=1))
    bqk = qkvw.tile([P, 4], F32)
    bv = qkvw.tile([1, NHL * HD], F32)
    bv_h = qkvw.tile([1, NHL * HD], F16)
    wq = qkvw.tile([P, KO, NHL * HD], F16)
    wk = qkvw.tile([P, KO, NHL * HD], F16)
    wv = qkvw.tile([P, KO, NHL * HD], F16)

    def emit_qkv_loads():
        # scalar HWDGE queue: runs concurrently with the x loads on sync
        nc.scalar.dma_start(bqk[:], bqk_in[:])
        nc.scalar.dma_start(bv[:], bv_in[:])
        nc.vector.tensor_copy(bv_h[:], bv[:])
        nc.scalar.dma_start(wq[:], wq_in[:])
        nc.scalar.dma_start(wk[:], wk_in[:])
        nc.scalar.dma_start(wv[:], wv_in[:])
    xnTp = p12.enter_context(tc.tile_pool(name="xnTp", bufs=2))
    xtiles = p12.enter_context(tc.tile_pool(name="xtiles", bufs=3))

    xnTs = {}
    ctxbs = {}
    xload = [None]

    def emit_p1(tb):
        """x load + LN1 + XBAR transpose for 4 tiles of band tb.

        All 4 x loads are emitted before any XBAR so the sync queue's
        in-order dispatch never parks a load behind an XBAR's z-wait.
        For bands 0-1 (latency-critical, before the scalar engine starts
        the softmax exps) each z transpose is split in half across the two
        HWDGE queues; bands 2-3 go as single dispatches on sync so the
        scalar queue stays free for exps."""
        xnT = xnTp.tile([P, KO, 512], F16, tag="xnT")
        xnTs[tb] = xnT
        zs = []
        for tt4 in range(4):
            tt = 4 * tb + tt4
            xt = xtiles.tile([P, C], F32, tag="x")
            xload[0] = nc.sync.dma_start(xt[:], x_in[tt * P:(tt + 1) * P, :])
            z = ztiles.tile([P, C], F16, tag="z")
            _layernorm_z(nc, stats, xt, z)
            zs.append(z)
            if tb == 0:
                # band 0 is startup-latency-critical and the PE is idle:
                # transpose on the PE (identity matmul) instead of the XBAR.
                # This also warms the HAM clock-gate early.
                ps16 = pssc.tile([P, 1024], F32, tag="sc",
                                 name="pstr").bitcast(F16)
                for ko in range(KO):
                    nc.tensor.matmul(
                        ps16[:, ko * P:(ko + 1) * P], z[:, ko * P:(ko + 1) * P],
                        ident[:], is_transpose=True, skip_group_check=True)
                    nc.vector.tensor_copy(
                        xnT[:, ko, tt4 * P:(tt4 + 1) * P],
                        ps16[:, ko * P:(ko + 1) * P])
        if tb > 0:
            for tt4 in range(4):
                nc.sync.dma_start_transpose(
                    xnT[:, 0:KO // 2, tt4 * P:(tt4 + 1) * P],
                    zs[tt4][:, 0:C // 2])
                nc.scalar.dma_start_transpose(
                    xnT[:, KO // 2:KO, tt4 * P:(tt4 + 1) * P],
                    zs[tt4][:, C // 2:C])

    def emit_w1_prefetch(lo, hi):
        # W1 prefetch chunks on the sync HWDGE queue; the hardware ring
        # semaphores throttle them behind earlier (x) loads automatically.
        for cchunk in range(lo, hi):
            nc.sync.dma_start(
                w1sb[:, 2 * cchunk:2 * cchunk + 2, :, :],
                w1_in[:, 2 * cchunk:2 * cchunk + 2, :, :])

    def emit_p2(tb):
        """QKV projections for band tb from xnT."""
        xnT = xnTs.pop(tb)
        for pp in range(2):
            for dst, w, bcol in ((qT2, wq, pp), (kT2, wk, 2 + pp)):
                ps = psbig.tile([P, 512], F32, tag="big")
                for ko in range(KO):
                    nc.tensor.matmul(
                        ps[:], w[:, ko, pp * P:(pp + 1) * P],
                        xnT[:, ko, :],
                        start=(ko == 0), stop=(ko == KO - 1))
                nc.vector.tensor_scalar_add(
                    dst[:, pp, tb * 512:(tb + 1) * 512], ps[:],
                    bqk[:, bcol:bcol + 1])
        for tt4 in range(4):
            tt = 4 * tb + tt4
            psf = pssc.tile([P, 1024], F32, tag="sc")
            ps = psf[:, 0:NHL * HD]
            nc.tensor.matmul(ps, ones1h[0:1, :], bv_h[0:1, :],
                             start=True, stop=False, skip_group_check=True)
            for ko in range(KO):
                nc.tensor.matmul(
                    ps, xnT[:, ko, tt4 * P:(tt4 + 1) * P], wv[:, ko, :],
                    start=False, stop=(ko == KO - 1),
                    skip_group_check=True)
            nc.vector.tensor_copy(
                v_sb[:, tt, :, 0:HD],
                ps.rearrange("p (h d) -> p h d", h=NHL))

    def emit_band(j, mid=None):
        """Attention band j (512 query rows), 4 local heads, then Wo.

        The two heads of a pair (hh=0 on partitions 0-63, hh=1 on 64-127)
        have their K=64 score matmuls emitted adjacently: the PE runs them
        concurrently on row-groups 0-63 / 64-127 (auto tile_position from
        base_partition), into different PSUM banks.

        ctx accumulates as [tokens, HD+1] per 128-token chunk, so softmax Z
        lands per-partition: the 1/Z normalize is a [P,1] reciprocal plus a
        [P,64] scale.  Normalized ctx pairs (2 heads = 128 cols) go back to
        the [head-cols, tokens] layout via XBAR transposes, alternating
        between the two HWDGE queues."""
        ctxb = bandp.tile([P, 2, 512], F16, tag="ctxb")
        assert j not in ctxbs
        for pp in range(2):
            pairT = pairp.tile([P, 4, P], F16, tag="pair")
            cqs = [psctxq.tile([P, 4, HD + 1], F32, tag="cq", name=f"cq{hh}")
                   for hh in range(2)]
            # paired off-diagonal blocks; hh=0/1 score MMs interleaved
            for pi in range(2 * j):
                sps2 = [pssc.tile([P, 1024], F32, tag="sc", name=f"sps{hh}")
                        for hh in range(2)]
                for half in range(2):
                    i = 2 * pi + half
                    for hh in range(2):
                        poff = 64 * hh
                        nc.tensor.matmul(
                            sps2[hh][:, half * 512:(half + 1) * 512],
                            kT2[poff:poff + HD, pp, i * P:(i + 1) * P],
                            qT2[poff:poff + HD, pp, j * 512:(j + 1) * 512],
                            start=True, stop=True)
                for hh in range(2):
                    pT = ptp.tile([P, 1024], F16, tag="pT")
                    nc.scalar.activation(pT[:], sps2[hh][:],
                                         mybir.ActivationFunctionType.Exp,
                                         scale=0.125)
                    for half in range(2):
                        i = 2 * pi + half
                        for tcc in range(4):
                            nc.tensor.matmul(
                                cqs[hh][:, tcc, :],
                                pT[:, half * 512 + tcc * P:
                                   half * 512 + (tcc + 1) * P],
                                v_sb[:, i, 2 * pp + hh, :],
                                start=(i == 0 and tcc == 0), stop=False,
                                skip_group_check=True)
            # diagonal blocks, live-column sliced; hh=1 uses cols 512:512+live
            # of the same [P,1024] psum tile (a different PSUM bank).
            for k in range(4):
                i = 4 * j + k
                lo = 128 * k
                live = 512 - lo
                sps = pssc.tile([P, 1024], F32, tag="sc")
                for hh in range(2):
                    poff = 64 * hh
                    nc.tensor.matmul(
                        sps[:, 512 * hh:512 * hh + live],
                        kT2[poff:poff + HD, pp, i * P:(i + 1) * P],
                        qT2[poff:poff + HD, pp,
                            j * 512 + lo:(j + 1) * 512],
                        start=True, stop=True, skip_group_check=True)
                for hh in range(2):
                    ms = mskp.tile([P, 512], F16, tag="ms")
                    nc.vector.scalar_tensor_tensor(
                        ms[:, 0:live], sps[:, 512 * hh:512 * hh + live],
                        0.125, masks[:, k, lo:512],
                        mybir.AluOpType.mult, mybir.AluOpType.add)
                    pT = ptp.tile([P, 1024], F16, tag="pT")
                    nc.scalar.activation(pT[:, 0:live], ms[:, 0:live],
                                         mybir.ActivationFunctionType.Exp)
                    for tcc in range(k, 4):
                        nc.tensor.matmul(
                            cqs[hh][:, tcc, :],
                            pT[:, (tcc - k) * P:(tcc - k + 1) * P],
                            v_sb[:, i, 2 * pp + hh, :],
                            start=(i == 0 and tcc == 0),
                            stop=(k == 3 and tcc == 3),
                            skip_group_check=True)
            # normalize per token chunk: [P,1] reciprocal + [P,64] scale
            for hh in range(2):
                poff = 64 * hh
                for tcc in range(4):
                    rzq = rzp.tile([P, 1], F32, tag="rzq")
                    nc.vector.reciprocal(rzq[:], cqs[hh][:, tcc, HD:HD + 1])
                    nc.vector.tensor_scalar_mul(
                        pairT[:, tcc, poff:poff + HD],
                        cqs[hh][:, tcc, 0:HD], rzq[:])
            # pairT -> ctxb transposes on the PE (identity matmul), writing
            # into the just-freed cq PSUM space; vector copies them out.
            for tcc in range(4):
                cqf = cqs[tcc % 2].rearrange("p a b -> p (a b)").bitcast(F16)
                slot = cqf[:, (tcc // 2) * 256:(tcc // 2) * 256 + P]
                nc.tensor.matmul(slot, pairT[:, tcc, :], ident[:],
                                 is_transpose=True, skip_group_check=True)
                nc.vector.tensor_copy(
                    ctxb[:, pp, tcc * P:(tcc + 1) * P], slot)
            if mid is not None and pp == 0:
                mid()
        ctxbs[j] = ctxb

    def emit_band_wo(j):
        # Wo partials for this band -> rs_ins[j]; stores on the sync queue
        ctxb = ctxbs.pop(j)
        for tt4 in range(4):
            stg = rstage.tile([P, C], F16, tag="stg")
            for cb in range(2):
                ops_ = psbig.tile([P, 512], F32, tag="big")
                for ch in range(2):
                    nc.tensor.matmul(
                        ops_[:],
                        ctxb[:, ch, tt4 * P:(tt4 + 1) * P],
                        wo[:, ch, cb * 512:(cb + 1) * 512],
                        start=(ch == 0), stop=(ch == 1))
                nc.vector.tensor_copy(stg[:, cb * 512:(cb + 1) * 512],
                                      ops_[:])
            # stores go on the (otherwise idle) gpsimd/SWDGE queue so they
            # complete fast and the RS doorbell behind them fires promptly
            nc.gpsimd.dma_start(
                rs_ins[j][tt4 * P:(tt4 + 1) * P, :], stg[:])

    def emit_rs(j):
        nc.gpsimd.collective_compute(
            "ReduceScatter", mybir.AluOpType.add,
            replica_groups=[[0, 1, 2, 3], [4, 5, 6, 7]],
            ins=[rs_ins[j].opt()], outs=[rs_outs[j].opt()],
        )

    def emit_p6(j):
        """x2 row block j = rs_out[j] + (xres+bo); LN2; transpose to xn2T."""
        rst = p6.tile([P, C], F16, tag="rst")
        nc.sync.dma_start(rst[:], rs_outs[j][:])
        xrt = p6.tile([P, C], F32, tag="xrt")
        nc.scalar.dma_start(xrt[:], xres_in[j * P:(j + 1) * P, :])
        nc.vector.tensor_tensor(x2[:, j, :], rst[:], xrt[:],
                                mybir.AluOpType.add)
        z2 = ztiles.tile([P, C], F16, tag="z")
        _layernorm_z(nc, stats, x2[:, j, :], z2)
        # both halves on sync: scalar must stay free for exps/relus
        nc.sync.dma_start_transpose(
            xn2T[:, 0:KO // 2, j * P:(j + 1) * P], z2[:, 0:C // 2])
        nc.sync.dma_start_transpose(
            xn2T[:, KO // 2:KO, j * P:(j + 1) * P], z2[:, C // 2:C])

    # --- P1/P2 + attention, interleaved -------------------------------------
    emit_qkv_loads()
    emit_p1(0)
    nc.scalar.dma_start(wo[:], wo_in[:])
    emit_p1(1)
    emit_p2(0)
    emit_p2(1)
    emit_p1(2)
    emit_band(0)
    emit_band_wo(0)
    emit_rs(0)
    emit_w1_prefetch(0, 4)
    emit_p2(2)
    emit_p1(3)
    emit_band(1)
    emit_band_wo(1)
    emit_rs(1)
    emit_w1_prefetch(4, 8)
    emit_p2(3)
    p12.close()
    # p6(0)/p6(1) are emitted mid-band, at points where their RS is
    # certainly complete, so their vector/scalar ops never head-of-line
    # block the attention work queued behind them.
    emit_band(2, mid=lambda: emit_p6(0))
    emit_band_wo(2)
    emit_rs(2)
    emit_w1_prefetch(8, 12)
    emit_band(3, mid=lambda: emit_p6(1))
    emit_band_wo(3)
    emit_rs(3)
    emit_w1_prefetch(12, 16)
    tc.no_sync_barrier()
    emit_p6(2)
    attnps2.close()

    # --- FFN -----------------------------------------------------------------
    with tc.tile_pool(name="w2p", bufs=12) as w2p, \
         tc.tile_pool(name="rp", bufs=2) as rp, \
         tc.tile_pool(name="psh1", bufs=2, space="PSUM") as psh1, \
         tc.tile_pool(name="psh2", bufs=4, space="PSUM") as psh2, \
         tc.tile_pool(name="yp", bufs=2) as yp:

        def ffn_half(th, mid_cbs=()):
            mid_cbs = dict(mid_cbs)
            h2ps = []
            for tt2 in range(2):
                for cb in range(2):
                    hp = psh2.tile([P, 512], F32, tag="h2")
                    nc.tensor.matmul(hp[:], ones1h[0:1, :],
                                     b2h[0:1, cb * 512:(cb + 1) * 512],
                                     start=True, stop=False,
                                     skip_group_check=True)
                    h2ps.append(hp)
            for ft in range(NFT):
                if ft in mid_cbs:
                    tc.no_sync_barrier()
                    mid_cbs[ft]()
                w2t = w2p.tile([P, C], F16, tag="w2")
                nc.sync.dma_start(w2t[:], w2_in[:, ft, :])
                h1 = psh1.tile([P, 256], F32, tag="h1")
                for ko in range(KO):
                    nc.tensor.matmul(h1[:], w1sb[:, ft, ko, :],
                                     xn2T[:, ko, th * 256:(th + 1) * 256],
                                     start=(ko == 0), stop=(ko == KO - 1))
                rT = rp.tile([P, 256], F16, tag="rT")
                # relu+bias on the scalar engine (idle during FFN)
                nc.scalar.activation(rT[:], h1[:],
                                     mybir.ActivationFunctionType.Relu,
                                     bias=b1p[:, ft:ft + 1])
                for tt2 in range(2):
                    for cb in range(2):
                        nc.tensor.matmul(
                            h2ps[2 * tt2 + cb][:],
                            rT[:, tt2 * P:(tt2 + 1) * P],
                            w2t[:, cb * 512:(cb + 1) * 512],
                            start=False, stop=(ft == NFT - 1),
                            skip_group_check=True)
            for tt2 in range(2):
                gt = 2 * th + tt2
                for cb in range(2):
                    yt = yp.tile([P, 512], F32, tag="y")
                    nc.vector.scalar_tensor_tensor(
                        yt[:], h2ps[2 * tt2 + cb][:], 1.0,
                        x2[:, gt, cb * 512:(cb + 1) * 512],
                        mybir.AluOpType.mult, mybir.AluOpType.add)
                    nc.sync.dma_start(
                        y_out[gt * P:(gt + 1) * P,
                              cb * 512:(cb + 1) * 512],
                        yt[:])

        ffn_half(0, mid_cbs={16: lambda: emit_p6(3)})
        ffn_half(1)

    attnps1.close()
    pctx.close()
    actx.close()
    ctx.close()


def _prep_inputs(x, Wq, Wk, Wv, Wo, bo, W1, b1, W2, b2, g1, be1, g2, be2):
    """Host-side sharding + layout packing. Returns list of 8 in_maps."""
    f32 = np.float32
    f16 = np.float16
    x = np.asarray(x, f32)
    Wq, Wk, Wv = (np.asarray(a, f32) for a in (Wq, Wk, Wv))
    Wo, bo = np.asarray(Wo, f32), np.asarray(bo, f32)
    W1, b1, W2, b2 = (np.asarray(a, f32) for a in (W1, b1, W2, b2))
    g1, be1, g2, be2 = (np.asarray(a, np.float64) for a in (g1, be1, g2, be2))

    def pack_qkv(W):  # [NHL, C, HD] g-folded -> [P, KO, NHL*HD] fp16
        Wl = (g1[None, :, None] * W.astype(np.float64)).astype(f32)
        flat = Wl.transpose(1, 0, 2).reshape(C, NHL * HD)   # [c, col]
        return np.ascontiguousarray(flat.reshape(KO, P, NHL * HD)
                                    .transpose(1, 0, 2)).astype(f16)

    # W1 folded with g2: [C, FF] -> [P, NFT, KO, P]
    W1f = (g2[:, None] * W1.astype(np.float64)).astype(f32)
    w1_arr = np.ascontiguousarray(
        W1f.reshape(KO, P, NFT, P).transpose(1, 2, 0, 3)).astype(f16)
    b1p = (b1.astype(np.float64) + be2 @ W1.astype(np.float64)).astype(f32)
    b1_arr = np.ascontiguousarray(b1p.reshape(NFT, P).T)
    w2_arr = np.ascontiguousarray(
        W2.reshape(NFT, P, C).transpose(1, 0, 2)).astype(f16)
    b2_arr = b2.reshape(1, C)

    in_maps = []
    for core in range(NCORES):
        b, r = divmod(core, TPG)
        hsel = slice(NHL * r, NHL * (r + 1))
        wq_arr = pack_qkv(Wq[hsel])
        wk_arr = pack_qkv(Wk[hsel])
        wv_arr = pack_qkv(Wv[hsel])
        # be1-induced biases (exact): col order = head-major within 256
        bq = (be1 @ Wq[hsel].astype(np.float64).transpose(1, 0, 2)
              .reshape(C, NHL * HD)).astype(f32)
        bk = (be1 @ Wk[hsel].astype(np.float64).transpose(1, 0, 2)
              .reshape(C, NHL * HD)).astype(f32)
        bvv = (be1 @ Wv[hsel].astype(np.float64).transpose(1, 0, 2)
               .reshape(C, NHL * HD)).astype(f32)
        bqk_arr = np.stack([bq[0:P], bq[P:2 * P], bk[0:P], bk[P:2 * P]],
                           axis=1).astype(f32)
        wo_arr = np.ascontiguousarray(
            Wo[NHL * HD * r: NHL * HD * (r + 1)].reshape(2, P, C)
            .transpose(1, 0, 2)).astype(f16)
        # per-band RS: core r owns rows 512j + 128r + [0,128) for band j
        li = np.arange(TLOC)
        lidx = 512 * (li // P) + P * r + (li % P)
        in_maps.append({
            "x": x[b],
            "xres": np.ascontiguousarray(x[b, lidx] + bo[None, :]),
            "wq": wq_arr, "wk": wk_arr, "wv": wv_arr,
            "bqk": bqk_arr, "bv": bvv.reshape(1, NHL * HD),
            "wo": wo_arr,
            "w1": w1_arr, "b1p": b1_arr, "w2": w2_arr, "b2": b2_arr,
        })
    return in_maps


def kernel(**inputs):
    global _CACHED_NC, LAST_RESULTS
    if _CACHED_NC is None:
        _CACHED_NC = _build_module()
    in_maps = _prep_inputs(**inputs)
    res = bass_utils.run_bass_kernel_spmd(
        _CACHED_NC, in_maps, core_ids=list(range(NCORES)))
    LAST_RESULTS = res
    y = np.empty((B, T, C), np.float32)
    li = np.arange(TLOC)
    lidx0 = 512 * (li // P) + (li % P)
    for core in range(NCORES):
        b, r = divmod(core, TPG)
        y[b, lidx0 + P * r] = res.results[core]["y"]
    return y
